# revision 15
# baseline (speedup 1.0000x reference)
"""GQA attention block on 8 Trainium2 cores.

Sharding: data-parallel over batch B=2 x tensor-parallel over the 4 KV groups
(cores 0-3 -> batch 0 groups 0-3, cores 4-7 -> batch 1 groups 0-3).
Each core computes Q/K/V projections for its group, attention for its 4 query
heads, and a row-sharded partial of the output projection.  The host sums the
4 partials per batch and adds the output bias.

v3 (vs v2 343 us, baseline 393 us):
- Host ships x and the weights pre-tiled partition-major so every transfer
  is one fat DMA with 4-16 KB contiguous per partition (the per-DMA issue
  overhead was gating phase A at ~200 GB/s with 1 KB lines).
- attnV delayed one more pipeline stage (exp(tg) -> scores(tg+1) ->
  attnV(tg-1)) so the PE never in-order-waits on the ACT exp.
- Denominator: accB merged into accA on the DVE, halving the ones-matmuls.
- Out-proj PSUM->SBUF copies all on the DVE (ACT is the phase-B floor).
- V transposes interleaved per chunk.
All matmul operands bf16 (same PE rate as fp32r, half the DMA/SBUF, 2x DVE);
K bias dropped (softmax-invariant), V bias folded into the host output bias.
"""
import sys

sys.path.insert(0, "/opt/trn_rl_repo")

import math
from collections import deque
from contextlib import ExitStack

import numpy as np
import ml_dtypes

import concourse.bacc as bacc
import concourse.tile as tile
import concourse.mybir as mybir
from concourse.bass_utils import run_bass_kernel_spmd
from concourse.masks import make_identity

F32 = mybir.dt.float32
F32R = mybir.dt.float32r
BF16 = mybir.dt.bfloat16
AF = mybir.ActivationFunctionType

D = 2048          # d_model
S = 2048          # sequence length
HD = 128          # head dim
R = 4             # q heads per kv group (on one core)
GD = R * HD       # 512: q-projection width per core
KT_TILES = S // 128   # 16 key-time tiles
KD_TILES = D // 128   # 16 contraction tiles for projections
N_SC = 4          # s-chunks of 512
SC = S // N_SC    # 512
SCALE = 1.0 / math.sqrt(HD)

_CACHED = {}


def _build():
    nc = bacc.Bacc("TRN2", target_bir_lowering=False, debug=False, num_devices=8)

    # all pre-tiled partition-major on the host for contiguous DMA
    XT = nc.dram_tensor("xt", [128, N_SC, KD_TILES, SC], BF16, kind="ExternalInput")
    WQ = nc.dram_tensor("wq", [128, KD_TILES, GD], BF16, kind="ExternalInput")
    WK = nc.dram_tensor("wk", [128, KD_TILES, HD], BF16, kind="ExternalInput")
    WV = nc.dram_tensor("wv", [128, KD_TILES, HD], BF16, kind="ExternalInput")
    WO = nc.dram_tensor("wo", [128, R, D], BF16, kind="ExternalInput")
    BQ = nc.dram_tensor("bq", [128, R], F32, kind="ExternalInput")
    OUT = nc.dram_tensor("out", [S, D], F32, kind="ExternalOutput")

    with tile.TileContext(nc) as tc, ExitStack() as ctx:
        # ---- long-lived tiles ----
        lp = ctx.enter_context(tc.tile_pool(name="long", bufs=1))
        qt_sb = lp.tile([128, R, S], BF16)        # Q^T per head: [dq, h, s]
        kt_sb = lp.tile([128, S], BF16)           # K^T: [dk, t]
        vt_sb = lp.tile([128, S], F32R)           # V^T: [dv, t]
        v_sb = lp.tile([128, KT_TILES, HD], BF16) # V natural: [t_sub, t_tile, dv]
        bq_sb = lp.tile([128, R], F32)
        ones_col = lp.tile([128, 1], BF16)
        ones_row = lp.tile([1, 128], F32R)
        ident = lp.tile([128, 128], F32R)

        nc.scalar.dma_start(bq_sb[:], BQ.ap())

        tmp_f = lp.tile([128, 128], F32)
        nc.gpsimd.memset(tmp_f[:], 1.0)
        nc.vector.tensor_copy(ones_col[:], tmp_f[:, 0:1])
        nc.vector.tensor_copy(ones_row[:], tmp_f[0:1, 0:128])
        make_identity(nc, tmp_f[:])
        nc.vector.tensor_copy(ident[:], tmp_f[:])

        # ---- phase A: projections ----
        with ExitStack() as actx:
            wp = actx.enter_context(tc.tile_pool(name="wqkv", bufs=1))
            xp = actx.enter_context(tc.tile_pool(name="xt", bufs=2))
            psa = actx.enter_context(tc.tile_pool(name="psa", bufs=4, space="PSUM"))
            pst = actx.enter_context(tc.tile_pool(name="pst", bufs=2, space="PSUM"))

            wq_sb = wp.tile([128, KD_TILES, GD], BF16)
            wk_sb = wp.tile([128, KD_TILES, HD], BF16)
            wv_sb = wp.tile([128, KD_TILES, HD], BF16)

            # fat contiguous DMAs; the weight loads issue on the ACT queue in
            # parallel with the x chunks on the sync queue
            nc.sync.dma_start(wk_sb[:], WK.ap())
            nc.scalar.dma_start(wv_sb[:], WV.ap())
            nc.scalar.dma_start(wq_sb[:], WQ.ap())

            for sc in range(N_SC):
                xt = xp.tile([128, KD_TILES, SC], BF16, tag="xt")
                if sc == 0:
                    # split so the k=0 matmuls start after ~1/4 chunk
                    for kg in range(4):
                        nc.sync.dma_start(
                            xt[:, kg * 4:(kg + 1) * 4, :],
                            XT.ap()[:, 0, kg * 4:(kg + 1) * 4, :],
                        )
                else:
                    nc.sync.dma_start(xt[:], XT.ap()[:, sc, :, :])
                # K^T (no bias: it cancels in the softmax)
                ps = psa.tile([128, SC], F32, tag="psa")
                for k in range(KD_TILES):
                    nc.tensor.matmul(
                        ps[:], lhsT=wk_sb[:, k, :], rhs=xt[:, k, :],
                        start=(k == 0), stop=(k == KD_TILES - 1),
                    )
                nc.vector.tensor_copy(kt_sb[:, sc * SC:(sc + 1) * SC], ps[:])
                # V^T (no bias: folded into the host-side output bias)
                ps = psa.tile([128, SC], F32, tag="psa")
                for k in range(KD_TILES):
                    nc.tensor.matmul(
                        ps[:], lhsT=wv_sb[:, k, :], rhs=xt[:, k, :],
                        start=(k == 0), stop=(k == KD_TILES - 1),
                    )
                nc.vector.tensor_copy(vt_sb[:, sc * SC:(sc + 1) * SC], ps[:])
                # Q^T for the 4 heads
                for dq in range(R):
                    ps = psa.tile([128, SC], F32, tag="psa")
                    for k in range(KD_TILES):
                        nc.tensor.matmul(
                            ps[:],
                            lhsT=wq_sb[:, k, dq * 128:(dq + 1) * 128],
                            rhs=xt[:, k, :],
                            start=(k == 0), stop=(k == KD_TILES - 1),
                        )
                    nc.scalar.activation(
                        qt_sb[:, dq, sc * SC:(sc + 1) * SC], ps[:],
                        AF.Identity, bias=bq_sb[:, dq:dq + 1],
                    )
                # V^T -> V natural for this chunk (4 PE transposes)
                for t in range(sc * 4, sc * 4 + 4):
                    pt_ps = pst.tile([128, 128], F32R, tag="pst")
                    nc.tensor.transpose(
                        pt_ps[:], vt_sb[:, t * 128:(t + 1) * 128], ident[:]
                    )
                    nc.vector.tensor_copy(v_sb[:, t, :], pt_ps[:])

        # ---- phase B: attention + out-proj ----
        with ExitStack() as bctx:
            wop = bctx.enter_context(tc.tile_pool(name="wo", bufs=1))
            wo_sb = wop.tile([128, R, D], BF16)
            nc.sync.dma_start(wo_sb[:], WO.ap())

            pss = bctx.enter_context(tc.tile_pool(name="pss", bufs=2, space="PSUM"))
            pso = bctx.enter_context(tc.tile_pool(name="pso", bufs=2, space="PSUM"))
            psm = bctx.enter_context(tc.tile_pool(name="psm", bufs=2, space="PSUM"))
            ptp = bctx.enter_context(tc.tile_pool(name="ptp", bufs=3))
            accp = bctx.enter_context(tc.tile_pool(name="accp", bufs=2))
            otp = bctx.enter_context(tc.tile_pool(name="otp", bufs=2))
            outp = bctx.enter_context(tc.tile_pool(name="outp", bufs=4))

            # closures emitted into the gaps of the tg loop; tails have
            # priority so PSUM ring slots (bufs=2) are always consumed
            # before their WAR reuse two heads later
            tail_aux = deque()
            op_aux = deque()       # holds (run_a, run_b) pairs
            pending = deque()      # forced continuation: run_b right after
                                   # its run_a so no tail can interleave a
                                   # psm allocation between the two halves

            def drain_aux(n=1):
                for _ in range(n):
                    if pending:
                        pending.popleft()()
                    elif tail_aux:
                        tail_aux.popleft()()
                    elif op_aux:
                        a, b = op_aux.popleft()
                        a()
                        pending.append(b)

            def tail_parts(h, ot_sb, ps_o, accA, accB):
                """Two closures: denominator+reciprocal, then broadcast+
                normalize (split so ps_b never queues behind the reciprocal)."""
                recip = accp.tile([1, SC], F32, tag="recip", name="recip")
                recip_r = accp.tile([1, SC], F32R, tag="recip_r", name="recip_r")

                def t1():
                    nc.vector.tensor_add(accA[:], accA[:], accB[:])
                    ps_d = psm.tile([1, SC], F32, tag="psm", name="ps_d")
                    for j in range(2):
                        nc.tensor.matmul(
                            ps_d[:], lhsT=ones_col[:], rhs=accA[:, j, :],
                            start=(j == 0), stop=(j == 1),
                        )
                    nc.vector.reciprocal_approx_fast(recip[:], ps_d[:])
                    nc.vector.tensor_copy(recip_r[:], recip[:])

                def t2():
                    ps_b = psm.tile([128, SC], F32, tag="psm", name="ps_b")
                    nc.tensor.matmul(
                        ps_b[:], lhsT=ones_row[:], rhs=recip_r[:],
                        start=True, stop=True,
                    )
                    bc = accp.tile([128, SC], F32, tag="bc", name="bc")
                    nc.scalar.copy(bc[:], ps_b[:])
                    nc.vector.tensor_mul(ot_sb[:, h, :], ps_o[:], bc[:])

                return t1, t2

            def out_proj_groups(sc, ot_sb, flush=False):
                """32 half-closures (2 matmuls each) so aux work interleaves
                smoothly instead of clumping into ACT-starving bursts.  The
                flush (trailing) block additionally rotates over the then-idle
                pso ring and alternates ACT/DVE copies so the PE does not
                stall on the 2-bank psm ring."""
                def group(st, oc, gi):
                    box = [None]

                    def run_a():
                        if flush and gi % 2 == 1:
                            ps_f = pso.tile([128, 512], F32, tag="pso",
                                            name="ps_f")
                        else:
                            ps_f = psm.tile([128, 512], F32, tag="psm",
                                            name="ps_f")
                        box[0] = ps_f
                        for dv in range(2):
                            nc.tensor.matmul(
                                ps_f[:],
                                lhsT=ot_sb[:, dv, st * 128:(st + 1) * 128],
                                rhs=wo_sb[:, dv, oc * 512:(oc + 1) * 512],
                                start=(dv == 0), stop=False,
                            )

                    def run_b():
                        ps_f = box[0]
                        for dv in range(2, R):
                            nc.tensor.matmul(
                                ps_f[:],
                                lhsT=ot_sb[:, dv, st * 128:(st + 1) * 128],
                                rhs=wo_sb[:, dv, oc * 512:(oc + 1) * 512],
                                start=False, stop=(dv == R - 1),
                            )
                        o_t = outp.tile([128, 512], F32, tag="out", name="o_t")
                        if flush and gi % 2 == 1:
                            nc.scalar.copy(o_t[:], ps_f[:])
                        else:
                            nc.vector.tensor_copy(o_t[:], ps_f[:])
                        nc.sync.dma_start(
                            OUT.ap()[
                                sc * SC + st * 128: sc * SC + (st + 1) * 128,
                                oc * 512:(oc + 1) * 512,
                            ],
                            o_t[:],
                        )
                    return run_a, run_b
                return [group(st, oc, st * (D // 512) + oc)
                        for st in range(SC // 128) for oc in range(D // 512)]

            # ---- one continuous software-pipelined stream over all
            # (chunk, head) units: exp(u,tg) -> scores(u,tg+1 or u+1,0) ->
            # attnV one stage behind.  No per-head fill/drain transients.
            units = [(sc, h) for sc in range(N_SC) for h in range(R)]
            ot_tiles = {}

            def unit_scores(u, tg):
                sc, h = units[u]
                ps_s = pss.tile([128, 2, SC], F32, tag="pss", name="ps_s")
                for i in range(2):
                    t = tg * 2 + i
                    nc.tensor.matmul(
                        ps_s[:, i, :],
                        lhsT=kt_sb[:, t * 128:(t + 1) * 128],
                        rhs=qt_sb[:, h, sc * SC:(sc + 1) * SC],
                        start=True, stop=True,
                    )
                return ps_s

            state = {}   # u -> dict(ps_o, accA, accB, pt_hist)
            prev = None  # (u, tg) of the attnV stage lagging one behind

            def emit_attnv(u, tg):
                st_ = state[u]
                pt = st_["pt_hist"][tg]
                for i in range(2):
                    t = tg * 2 + i
                    nc.tensor.matmul(
                        st_["ps_o"][:],
                        lhsT=v_sb[:, t, :],
                        rhs=pt[:, i, :],
                        start=(t == 0),
                        stop=(t == KT_TILES - 1),
                        skip_group_check=True,
                    )

            ps_s = unit_scores(0, 0)
            for u, (sc, h) in enumerate(units):
                if h == 0:
                    ot_tiles[sc] = otp.tile([128, R, SC], BF16, tag="ot",
                                            name="ot_sb")
                st_ = {
                    "ps_o": pso.tile([128, SC], F32, tag="pso", name="ps_o"),
                    "accA": accp.tile([128, 2, SC], BF16, tag="accA", name="accA"),
                    "accB": accp.tile([128, 2, SC], BF16, tag="accB", name="accB"),
                    "pt_hist": {},
                }
                state[u] = st_
                for tg in range(8):
                    if tg == 0:
                        pt = st_["accA"]
                    elif tg == 4:
                        pt = st_["accB"]
                    else:
                        pt = ptp.tile([128, 2, SC], BF16, tag="pt", name="pt")
                    nc.scalar.activation(pt[:], ps_s[:], AF.Exp, scale=SCALE)
                    st_["pt_hist"][tg] = pt
                    # next position's scores (rolls into the next unit)
                    if tg < 7:
                        ps_s = unit_scores(u, tg + 1)
                    elif u + 1 < len(units):
                        ps_s = unit_scores(u + 1, 0)
                    # attnV lags one stage: never in-order-waits on the ACT
                    if prev is not None:
                        emit_attnv(*prev)
                        if prev[1] == 7:
                            # previous unit complete: queue its tail
                            pu = prev[0]
                            psc, ph = units[pu]
                            t1, t2 = tail_parts(
                                ph, ot_tiles[psc], state[pu]["ps_o"],
                                state[pu]["accA"], state[pu]["accB"],
                            )
                            tail_aux.append(t1)
                            tail_aux.append(t2)
                            del state[pu]["pt_hist"]
                    # denominator partials (wide bf16 adds, 2x DVE mode)
                    if tg in (1, 2, 3):
                        nc.vector.tensor_add(st_["accA"][:], st_["accA"][:],
                                             pt[:])
                    elif tg in (5, 6, 7):
                        nc.vector.tensor_add(st_["accB"][:], st_["accB"][:],
                                             pt[:])
                    prev = (u, tg)
                    drain_aux(2 if tg in (2, 5) else 1)
                if h == R - 1:
                    op_aux.extend(out_proj_groups(sc, ot_tiles[sc],
                                                  flush=(sc == N_SC - 1)))
            # flush: the trailing attnV, last tail, last chunk's out-proj
            emit_attnv(*prev)
            sc, h = units[-1]
            t1, t2 = tail_parts(h, ot_tiles[sc], state[len(units) - 1]["ps_o"],
                                state[len(units) - 1]["accA"],
                                state[len(units) - 1]["accB"])
            tail_aux.append(t1)
            tail_aux.append(t2)
            while pending or tail_aux or op_aux:
                drain_aux()

    nc.compile()
    return nc


def _get_nc():
    if "nc" not in _CACHED:
        _CACHED["nc"] = _build()
    return _CACHED["nc"]


def _tile_p(a, nt, width):
    """[nt*128, width] -> [128, nt, width] partition-major."""
    return np.ascontiguousarray(
        a.reshape(nt, 128, width).transpose(1, 0, 2)
    )


def _make_in_maps(x, Wq, bq, Wk, Wv, Wo):
    bf = ml_dtypes.bfloat16
    in_maps = []
    xts = []
    for b in range(2):
        xt = np.ascontiguousarray(x[b].T).astype(bf)      # [D, S]
        # [D, S] -> [128, N_SC, KD, SC]
        xt = xt.reshape(KD_TILES, 128, N_SC, SC).transpose(1, 2, 0, 3)
        xts.append(np.ascontiguousarray(xt))
    wq_b = Wq.astype(bf)
    wk_b = Wk.astype(bf)
    wv_b = Wv.astype(bf)
    wo_b = Wo.astype(bf)
    for core in range(8):
        b, g = divmod(core, 4)
        in_maps.append({
            "xt": xts[b],
            "wq": _tile_p(wq_b[:, g * GD:(g + 1) * GD], KD_TILES, GD),
            "wk": _tile_p(wk_b[:, g * HD:(g + 1) * HD], KD_TILES, HD),
            "wv": _tile_p(wv_b[:, g * HD:(g + 1) * HD], KD_TILES, HD),
            "wo": _tile_p(wo_b[g * GD:(g + 1) * GD, :], R, D),
            "bq": np.ascontiguousarray(
                bq[g * GD:(g + 1) * GD].reshape(R, 128).T
            ).astype(np.float32),
        })
    return in_maps


def kernel(x, Wq, bq, Wk, bk, Wv, bv, Wo, bo, _trace=False):
    x = np.asarray(x, dtype=np.float32)
    Wq = np.asarray(Wq, np.float32)
    bq = np.asarray(bq, np.float32)
    Wk = np.asarray(Wk, np.float32)
    Wv = np.asarray(Wv, np.float32)
    Wo = np.asarray(Wo, np.float32)
    bv = np.asarray(bv, np.float32)
    bo = np.asarray(bo, np.float32)
    nc = _get_nc()
    in_maps = _make_in_maps(x, Wq, bq, Wk, Wv, Wo)
    res = run_bass_kernel_spmd(nc, in_maps, list(range(8)), trace=_trace)
    # host-side bias: bo + contribution of the V bias through the out-proj
    # (attention weights sum to 1, so each head adds bv[group] @ Wo_head)
    H = 16
    bias_full = bo.copy()
    for h in range(H):
        g = h // R
        bias_full += bv[g * HD:(g + 1) * HD] @ Wo[h * HD:(h + 1) * HD, :]
    out = np.empty((2, S, D), np.float32)
    for b in range(2):
        acc = res.results[b * 4]["out"].astype(np.float32)
        for g in range(1, 4):
            acc = acc + res.results[b * 4 + g]["out"]
        out[b] = acc + bias_full[None, :]
    if _trace:
        return out, res
    return out


# revision 16
# speedup vs baseline: 1.0133x; 1.0133x over previous
"""GQA attention block on 8 Trainium2 cores.

Sharding: data-parallel over batch B=2 x tensor-parallel over the 4 KV groups
(cores 0-3 -> batch 0 groups 0-3, cores 4-7 -> batch 1 groups 0-3).
Each core computes Q/K/V projections for its group, attention for its 4 query
heads, and a row-sharded partial of the output projection.  The host sums the
4 partials per batch and adds the output bias.

v3 (vs v2 343 us, baseline 393 us):
- Host ships x and the weights pre-tiled partition-major so every transfer
  is one fat DMA with 4-16 KB contiguous per partition (the per-DMA issue
  overhead was gating phase A at ~200 GB/s with 1 KB lines).
- attnV delayed one more pipeline stage (exp(tg) -> scores(tg+1) ->
  attnV(tg-1)) so the PE never in-order-waits on the ACT exp.
- Denominator: accB merged into accA on the DVE, halving the ones-matmuls.
- Out-proj PSUM->SBUF copies all on the DVE (ACT is the phase-B floor).
- V transposes interleaved per chunk.
All matmul operands bf16 (same PE rate as fp32r, half the DMA/SBUF, 2x DVE);
K bias dropped (softmax-invariant), V bias folded into the host output bias.
"""
import sys

sys.path.insert(0, "/opt/trn_rl_repo")

import math
from collections import deque
from contextlib import ExitStack

import numpy as np
import ml_dtypes

import concourse.bacc as bacc
import concourse.tile as tile
import concourse.mybir as mybir
from concourse.bass_utils import run_bass_kernel_spmd
from concourse.masks import make_identity

F32 = mybir.dt.float32
F32R = mybir.dt.float32r
BF16 = mybir.dt.bfloat16
AF = mybir.ActivationFunctionType

D = 2048          # d_model
S = 2048          # sequence length
HD = 128          # head dim
R = 4             # q heads per kv group (on one core)
GD = R * HD       # 512: q-projection width per core
KT_TILES = S // 128   # 16 key-time tiles
KD_TILES = D // 128   # 16 contraction tiles for projections
N_SC = 4          # s-chunks of 512
SC = S // N_SC    # 512
SCALE = 1.0 / math.sqrt(HD)

_CACHED = {}


def _build():
    nc = bacc.Bacc("TRN2", target_bir_lowering=False, debug=False, num_devices=8)

    # all pre-tiled partition-major on the host for contiguous DMA
    XT = nc.dram_tensor("xt", [128, N_SC, KD_TILES, SC], BF16, kind="ExternalInput")
    WQ = nc.dram_tensor("wq", [128, KD_TILES, GD], BF16, kind="ExternalInput")
    WK = nc.dram_tensor("wk", [128, KD_TILES, HD], BF16, kind="ExternalInput")
    WV = nc.dram_tensor("wv", [128, KD_TILES, HD], BF16, kind="ExternalInput")
    WO = nc.dram_tensor("wo", [128, R, D], BF16, kind="ExternalInput")
    BQ = nc.dram_tensor("bq", [128, R], F32, kind="ExternalInput")
    OUT = nc.dram_tensor("out", [S, D], F32, kind="ExternalOutput")

    with tile.TileContext(nc) as tc, ExitStack() as ctx:
        # ---- long-lived tiles ----
        lp = ctx.enter_context(tc.tile_pool(name="long", bufs=1))
        qt_sb = lp.tile([128, R, S], BF16)        # Q^T per head: [dq, h, s]
        kt_sb = lp.tile([128, S], BF16)           # K^T: [dk, t]
        vt_sb = lp.tile([128, S], F32R)           # V^T: [dv, t]
        v_sb = lp.tile([128, KT_TILES, HD], BF16) # V natural: [t_sub, t_tile, dv]
        bq_sb = lp.tile([128, R], F32)
        ones_col = lp.tile([128, 1], BF16)
        ones_row = lp.tile([1, 128], F32R)
        ident = lp.tile([128, 128], F32R)

        nc.scalar.dma_start(bq_sb[:], BQ.ap())

        tmp_f = lp.tile([128, 128], F32)
        nc.gpsimd.memset(tmp_f[:], 1.0)
        nc.vector.tensor_copy(ones_col[:], tmp_f[:, 0:1])
        nc.vector.tensor_copy(ones_row[:], tmp_f[0:1, 0:128])
        make_identity(nc, tmp_f[:])
        nc.vector.tensor_copy(ident[:], tmp_f[:])

        # ---- phase A: projections ----
        with ExitStack() as actx:
            wp = actx.enter_context(tc.tile_pool(name="wqkv", bufs=1))
            xp = actx.enter_context(tc.tile_pool(name="xt", bufs=2))
            psa = actx.enter_context(tc.tile_pool(name="psa", bufs=4, space="PSUM"))
            pst = actx.enter_context(tc.tile_pool(name="pst", bufs=2, space="PSUM"))

            wq_sb = wp.tile([128, KD_TILES, GD], BF16)
            wk_sb = wp.tile([128, KD_TILES, HD], BF16)
            wv_sb = wp.tile([128, KD_TILES, HD], BF16)

            # fat contiguous DMAs; K/V weights first so their matmuls can
            # chase the first x chunk
            nc.sync.dma_start(wk_sb[:], WK.ap())
            nc.sync.dma_start(wv_sb[:], WV.ap())

            for sc in range(N_SC):
                xt = xp.tile([128, KD_TILES, SC], BF16, tag="xt")
                if sc == 0:
                    # split so the k=0 matmuls start after ~1/4 chunk
                    for kg in range(4):
                        nc.sync.dma_start(
                            xt[:, kg * 4:(kg + 1) * 4, :],
                            XT.ap()[:, 0, kg * 4:(kg + 1) * 4, :],
                        )
                    nc.sync.dma_start(wq_sb[:], WQ.ap())
                else:
                    nc.sync.dma_start(xt[:], XT.ap()[:, sc, :, :])
                # K^T (no bias: it cancels in the softmax)
                ps = psa.tile([128, SC], F32, tag="psa")
                for k in range(KD_TILES):
                    nc.tensor.matmul(
                        ps[:], lhsT=wk_sb[:, k, :], rhs=xt[:, k, :],
                        start=(k == 0), stop=(k == KD_TILES - 1),
                    )
                nc.vector.tensor_copy(kt_sb[:, sc * SC:(sc + 1) * SC], ps[:])
                # V^T (no bias: folded into the host-side output bias)
                ps = psa.tile([128, SC], F32, tag="psa")
                for k in range(KD_TILES):
                    nc.tensor.matmul(
                        ps[:], lhsT=wv_sb[:, k, :], rhs=xt[:, k, :],
                        start=(k == 0), stop=(k == KD_TILES - 1),
                    )
                nc.vector.tensor_copy(vt_sb[:, sc * SC:(sc + 1) * SC], ps[:])
                # Q^T for the 4 heads
                for dq in range(R):
                    ps = psa.tile([128, SC], F32, tag="psa")
                    for k in range(KD_TILES):
                        nc.tensor.matmul(
                            ps[:],
                            lhsT=wq_sb[:, k, dq * 128:(dq + 1) * 128],
                            rhs=xt[:, k, :],
                            start=(k == 0), stop=(k == KD_TILES - 1),
                        )
                    nc.scalar.activation(
                        qt_sb[:, dq, sc * SC:(sc + 1) * SC], ps[:],
                        AF.Identity, bias=bq_sb[:, dq:dq + 1],
                    )
                # V^T -> V natural for this chunk (4 PE transposes)
                for t in range(sc * 4, sc * 4 + 4):
                    pt_ps = pst.tile([128, 128], F32R, tag="pst")
                    nc.tensor.transpose(
                        pt_ps[:], vt_sb[:, t * 128:(t + 1) * 128], ident[:]
                    )
                    nc.vector.tensor_copy(v_sb[:, t, :], pt_ps[:])

        # ---- phase B: attention + out-proj ----
        with ExitStack() as bctx:
            wop = bctx.enter_context(tc.tile_pool(name="wo", bufs=1))
            wo_sb = wop.tile([128, R, D], BF16)
            nc.sync.dma_start(wo_sb[:], WO.ap())

            pss = bctx.enter_context(tc.tile_pool(name="pss", bufs=2, space="PSUM"))
            pso = bctx.enter_context(tc.tile_pool(name="pso", bufs=2, space="PSUM"))
            psm = bctx.enter_context(tc.tile_pool(name="psm", bufs=2, space="PSUM"))
            ptp = bctx.enter_context(tc.tile_pool(name="ptp", bufs=3))
            accp = bctx.enter_context(tc.tile_pool(name="accp", bufs=2))
            otp = bctx.enter_context(tc.tile_pool(name="otp", bufs=2))
            outp = bctx.enter_context(tc.tile_pool(name="outp", bufs=4))

            # closures emitted into the gaps of the tg loop; tails have
            # priority so PSUM ring slots (bufs=2) are always consumed
            # before their WAR reuse two heads later
            tail_aux = deque()
            op_aux = deque()       # holds (run_a, run_b) pairs
            pending = deque()      # forced continuation: run_b right after
                                   # its run_a so no tail can interleave a
                                   # psm allocation between the two halves

            def drain_aux(n=1):
                for _ in range(n):
                    if pending:
                        pending.popleft()()
                    elif tail_aux:
                        tail_aux.popleft()()
                    elif op_aux:
                        a, b = op_aux.popleft()
                        a()
                        pending.append(b)

            def tail_parts(h, ot_sb, ps_o, accA, accB):
                """Two closures: denominator+reciprocal, then broadcast+
                normalize (split so ps_b never queues behind the reciprocal)."""
                recip = accp.tile([1, SC], F32, tag="recip", name="recip")
                recip_r = accp.tile([1, SC], F32R, tag="recip_r", name="recip_r")

                def t1():
                    nc.vector.tensor_add(accA[:], accA[:], accB[:])
                    ps_d = psm.tile([1, SC], F32, tag="psm", name="ps_d")
                    for j in range(2):
                        nc.tensor.matmul(
                            ps_d[:], lhsT=ones_col[:], rhs=accA[:, j, :],
                            start=(j == 0), stop=(j == 1),
                        )
                    nc.vector.reciprocal_approx_fast(recip[:], ps_d[:])
                    nc.vector.tensor_copy(recip_r[:], recip[:])

                def t2():
                    ps_b = psm.tile([128, SC], F32, tag="psm", name="ps_b")
                    nc.tensor.matmul(
                        ps_b[:], lhsT=ones_row[:], rhs=recip_r[:],
                        start=True, stop=True,
                    )
                    bc = accp.tile([128, SC], F32, tag="bc", name="bc")
                    nc.scalar.copy(bc[:], ps_b[:])
                    nc.vector.tensor_mul(ot_sb[:, h, :], ps_o[:], bc[:])

                return t1, t2

            def out_proj_groups(sc, ot_sb, flush=False):
                """32 half-closures (2 matmuls each) so aux work interleaves
                smoothly instead of clumping into ACT-starving bursts.  The
                flush (trailing) block additionally rotates over the then-idle
                pso ring and alternates ACT/DVE copies so the PE does not
                stall on the 2-bank psm ring."""
                def group(st, oc, gi):
                    box = [None]

                    def run_a():
                        if flush and gi % 2 == 1:
                            ps_f = pso.tile([128, 512], F32, tag="pso",
                                            name="ps_f")
                        else:
                            ps_f = psm.tile([128, 512], F32, tag="psm",
                                            name="ps_f")
                        box[0] = ps_f
                        for dv in range(2):
                            nc.tensor.matmul(
                                ps_f[:],
                                lhsT=ot_sb[:, dv, st * 128:(st + 1) * 128],
                                rhs=wo_sb[:, dv, oc * 512:(oc + 1) * 512],
                                start=(dv == 0), stop=False,
                            )

                    def run_b():
                        ps_f = box[0]
                        for dv in range(2, R):
                            nc.tensor.matmul(
                                ps_f[:],
                                lhsT=ot_sb[:, dv, st * 128:(st + 1) * 128],
                                rhs=wo_sb[:, dv, oc * 512:(oc + 1) * 512],
                                start=False, stop=(dv == R - 1),
                            )
                        o_t = outp.tile([128, 512], F32, tag="out", name="o_t")
                        if flush and gi % 2 == 1:
                            nc.scalar.copy(o_t[:], ps_f[:])
                        else:
                            nc.vector.tensor_copy(o_t[:], ps_f[:])
                        nc.sync.dma_start(
                            OUT.ap()[
                                sc * SC + st * 128: sc * SC + (st + 1) * 128,
                                oc * 512:(oc + 1) * 512,
                            ],
                            o_t[:],
                        )
                    return run_a, run_b
                return [group(st, oc, st * (D // 512) + oc)
                        for st in range(SC // 128) for oc in range(D // 512)]

            # ---- one continuous software-pipelined stream over all
            # (chunk, head) units: exp(u,tg) -> scores(u,tg+1 or u+1,0) ->
            # attnV one stage behind.  No per-head fill/drain transients.
            units = [(sc, h) for sc in range(N_SC) for h in range(R)]
            ot_tiles = {}

            def unit_scores(u, tg):
                sc, h = units[u]
                ps_s = pss.tile([128, 2, SC], F32, tag="pss", name="ps_s")
                for i in range(2):
                    t = tg * 2 + i
                    nc.tensor.matmul(
                        ps_s[:, i, :],
                        lhsT=kt_sb[:, t * 128:(t + 1) * 128],
                        rhs=qt_sb[:, h, sc * SC:(sc + 1) * SC],
                        start=True, stop=True,
                    )
                return ps_s

            state = {}   # u -> dict(ps_o, accA, accB, pt_hist)
            prev = None  # (u, tg) of the attnV stage lagging one behind

            def emit_attnv(u, tg):
                st_ = state[u]
                pt = st_["pt_hist"][tg]
                for i in range(2):
                    t = tg * 2 + i
                    nc.tensor.matmul(
                        st_["ps_o"][:],
                        lhsT=v_sb[:, t, :],
                        rhs=pt[:, i, :],
                        start=(t == 0),
                        stop=(t == KT_TILES - 1),
                        skip_group_check=True,
                    )

            ps_s = unit_scores(0, 0)
            for u, (sc, h) in enumerate(units):
                if h == 0:
                    ot_tiles[sc] = otp.tile([128, R, SC], BF16, tag="ot",
                                            name="ot_sb")
                st_ = {
                    "ps_o": pso.tile([128, SC], F32, tag="pso", name="ps_o"),
                    "accA": accp.tile([128, 2, SC], BF16, tag="accA", name="accA"),
                    "accB": accp.tile([128, 2, SC], BF16, tag="accB", name="accB"),
                    "pt_hist": {},
                }
                state[u] = st_
                for tg in range(8):
                    if tg == 0:
                        pt = st_["accA"]
                    elif tg == 4:
                        pt = st_["accB"]
                    else:
                        pt = ptp.tile([128, 2, SC], BF16, tag="pt", name="pt")
                    nc.scalar.activation(pt[:], ps_s[:], AF.Exp, scale=SCALE)
                    st_["pt_hist"][tg] = pt
                    # next position's scores (rolls into the next unit)
                    if tg < 7:
                        ps_s = unit_scores(u, tg + 1)
                    elif u + 1 < len(units):
                        ps_s = unit_scores(u + 1, 0)
                    # attnV lags one stage: never in-order-waits on the ACT
                    if prev is not None:
                        emit_attnv(*prev)
                        if prev[1] == 7:
                            # previous unit complete: queue its tail
                            pu = prev[0]
                            psc, ph = units[pu]
                            t1, t2 = tail_parts(
                                ph, ot_tiles[psc], state[pu]["ps_o"],
                                state[pu]["accA"], state[pu]["accB"],
                            )
                            tail_aux.append(t1)
                            tail_aux.append(t2)
                            del state[pu]["pt_hist"]
                    # denominator partials (wide bf16 adds, 2x DVE mode)
                    if tg in (1, 2, 3):
                        nc.vector.tensor_add(st_["accA"][:], st_["accA"][:],
                                             pt[:])
                    elif tg in (5, 6, 7):
                        nc.vector.tensor_add(st_["accB"][:], st_["accB"][:],
                                             pt[:])
                    prev = (u, tg)
                    drain_aux(2 if tg in (2, 5) else 1)
                if h == R - 1:
                    op_aux.extend(out_proj_groups(sc, ot_tiles[sc],
                                                  flush=(sc == N_SC - 1)))
            # flush: the trailing attnV, last tail, last chunk's out-proj
            emit_attnv(*prev)
            sc, h = units[-1]
            t1, t2 = tail_parts(h, ot_tiles[sc], state[len(units) - 1]["ps_o"],
                                state[len(units) - 1]["accA"],
                                state[len(units) - 1]["accB"])
            tail_aux.append(t1)
            tail_aux.append(t2)
            while pending or tail_aux or op_aux:
                drain_aux()

    nc.compile()
    return nc


def _get_nc():
    if "nc" not in _CACHED:
        _CACHED["nc"] = _build()
    return _CACHED["nc"]


def _tile_p(a, nt, width):
    """[nt*128, width] -> [128, nt, width] partition-major."""
    return np.ascontiguousarray(
        a.reshape(nt, 128, width).transpose(1, 0, 2)
    )


def _make_in_maps(x, Wq, bq, Wk, Wv, Wo):
    bf = ml_dtypes.bfloat16
    in_maps = []
    xts = []
    for b in range(2):
        xt = np.ascontiguousarray(x[b].T).astype(bf)      # [D, S]
        # [D, S] -> [128, N_SC, KD, SC]
        xt = xt.reshape(KD_TILES, 128, N_SC, SC).transpose(1, 2, 0, 3)
        xts.append(np.ascontiguousarray(xt))
    wq_b = Wq.astype(bf)
    wk_b = Wk.astype(bf)
    wv_b = Wv.astype(bf)
    wo_b = Wo.astype(bf)
    for core in range(8):
        b, g = divmod(core, 4)
        in_maps.append({
            "xt": xts[b],
            "wq": _tile_p(wq_b[:, g * GD:(g + 1) * GD], KD_TILES, GD),
            "wk": _tile_p(wk_b[:, g * HD:(g + 1) * HD], KD_TILES, HD),
            "wv": _tile_p(wv_b[:, g * HD:(g + 1) * HD], KD_TILES, HD),
            "wo": _tile_p(wo_b[g * GD:(g + 1) * GD, :], R, D),
            "bq": np.ascontiguousarray(
                bq[g * GD:(g + 1) * GD].reshape(R, 128).T
            ).astype(np.float32),
        })
    return in_maps


def kernel(x, Wq, bq, Wk, bk, Wv, bv, Wo, bo, _trace=False):
    x = np.asarray(x, dtype=np.float32)
    Wq = np.asarray(Wq, np.float32)
    bq = np.asarray(bq, np.float32)
    Wk = np.asarray(Wk, np.float32)
    Wv = np.asarray(Wv, np.float32)
    Wo = np.asarray(Wo, np.float32)
    bv = np.asarray(bv, np.float32)
    bo = np.asarray(bo, np.float32)
    nc = _get_nc()
    in_maps = _make_in_maps(x, Wq, bq, Wk, Wv, Wo)
    res = run_bass_kernel_spmd(nc, in_maps, list(range(8)), trace=_trace)
    # host-side bias: bo + contribution of the V bias through the out-proj
    # (attention weights sum to 1, so each head adds bv[group] @ Wo_head)
    H = 16
    bias_full = bo.copy()
    for h in range(H):
        g = h // R
        bias_full += bv[g * HD:(g + 1) * HD] @ Wo[h * HD:(h + 1) * HD, :]
    out = np.empty((2, S, D), np.float32)
    for b in range(2):
        acc = res.results[b * 4]["out"].astype(np.float32)
        for g in range(1, 4):
            acc = acc + res.results[b * 4 + g]["out"]
        out[b] = acc + bias_full[None, :]
    if _trace:
        return out, res
    return out
